# revision 4
# baseline (speedup 1.0000x reference)
"""Trainium2 Bass kernel for BatchGraphConv (GNN message passing).

out = relu(segment_sum(adj_vals * (x@W+b)[edge_src], edge_dst))
    = relu(agg @ W + deg * b)   where agg[i] = sum_e v_e x[src_e], deg[i] = sum_e v_e

Sharding: destination nodes split across 8 cores (12500 each). Each core:
  - hardware dma_gather of x rows for its edges (grouped by 128-node dst
    block and 25000-row src chunk so indices fit int16)
  - per 128-edge tile: P = (iota == r) * v  (value-weighted one-hot, DVE)
  - TensorE: psum_block += P^T @ G  (segment sum into 128-node block)
  - epilogue per block: transpose, @W, (+deg*b), relu, transpose, DMA out.
Host does index bookkeeping only (sort/group/pad); all FLOPs on device.
"""

import os
import sys
import time

import numpy as np

for _p in ("/opt/trn_rl_repo", "/root/.axon_site/_ro/trn_rl_repo"):
    if os.path.isdir(_p) and _p not in sys.path:
        sys.path.insert(0, _p)


class CFG:
    N = 100000
    E = 1600000
    D = 64
    NCORES = 8
    NS = 12500          # dst nodes per core
    BLK = 128           # max nodes per block (= PSUM partitions)
    NCHUNK = 4          # src index windows
    CW = 25000          # src chunk width (int16-addressable rows)
    SB_BLOCKS = 8       # blocks per superblock (gather batch)
    MAX_GATHER = 1024   # max indices per dma_gather instruction (HW limit)
    QSLOTS = 512        # slots per (block, chunk); multiple of 128
    P_ACT_EVERY = 0     # 0=off; else every k-th P-build goes to ScalarE


def _ceil_to(a, m):
    return -(-a // m) * m


def _prepare(cfg, adj_vals, edge_src, edge_dst):
    """Host-side index prep. Returns (meta, per_core_arrays)."""
    NC, NS, BLK, NB, NCH, CW = (
        cfg.NCORES, cfg.NS, cfg.BLK, cfg.NB, cfg.NCHUNK, cfg.CW)
    nreg = NB * NCH

    core_of = edge_dst // NS
    per_core_raw = []
    counts = np.zeros((NC, NB, NCH), np.int64)
    for m in range(NC):
        sel = np.nonzero(core_of == m)[0]
        ldst = edge_dst[sel] - m * NS
        blk = ldst // BLK
        r = (ldst % BLK).astype(np.float32)
        ch = edge_src[sel] // CW
        srcrel = (edge_src[sel] - ch * CW).astype(np.int16)
        key = blk * NCH + ch
        order = np.argsort(key, kind="stable")
        key = key[order]
        starts = np.searchsorted(key, np.arange(nreg + 1))
        counts[m] = np.diff(starts).reshape(NB, NCH)
        per_core_raw.append((srcrel[order], r[order],
                             adj_vals[sel[order]].astype(np.float32), starts))

    # padded region sizes, shared across cores
    Q = np.array([[_ceil_to(int(c), 128) if c else 0
                   for c in row] for row in counts.max(axis=0)], np.int64)
    # ensure every block has at least one tile (so psum is initialized)
    for b in range(NB):
        if Q[b].sum() == 0:
            Q[b, 0] = 128

    # superblocks
    sb_list = [list(range(s, min(s + cfg.SB_BLOCKS, NB)))
               for s in range(0, NB, cfg.SB_BLOCKS)]

    # global slot layout: for sb -> for c -> for b in sb
    slot_off = 0
    regions = {}          # (b, c) -> global slot offset
    sb_meta = []          # per sb: {c: (slots, off_slots)}
    for blocks in sb_list:
        cmeta = {}
        for c in range(NCH):
            off_c = slot_off
            for b in blocks:
                regions[(b, c)] = slot_off
                slot_off += int(Q[b, c])
            cmeta[c] = (slot_off - off_c, off_c)
        sb_meta.append({"blocks": blocks, "chunks": cmeta})
    TOT = slot_off
    assert TOT % 128 == 0

    # per-block matmul sequence: (c, tile col within (sb,c) buffer, global tile)
    blk_seq = [[] for _ in range(NB)]
    for sbi, blocks in enumerate(sb_list):
        for c in range(NCH):
            _, off_c = sb_meta[sbi]["chunks"][c]
            for b in blocks:
                roff = regions[(b, c)]
                for t in range(int(Q[b, c]) // 128):
                    blk_seq[b].append(
                        (c, (roff - off_c) // 128 + t, roff // 128 + t))

    meta = {"Q": Q, "sb_meta": sb_meta, "blk_seq": blk_seq, "TOT": TOT}

    # per-core slot arrays
    per_core = []
    for m in range(NC):
        srcrel, r, v, starts = per_core_raw[m]
        idx_all = np.zeros(TOT, np.int16)
        r_all = np.zeros(TOT, np.float32)
        v_all = np.zeros(TOT, np.float32)
        for b in range(NB):
            for c in range(NCH):
                cnt = int(counts[m, b, c])
                if cnt == 0:
                    continue
                s0 = starts[b * NCH + c]
                d0 = regions[(b, c)]
                idx_all[d0:d0 + cnt] = srcrel[s0:s0 + cnt]
                r_all[d0:d0 + cnt] = r[s0:s0 + cnt]
                v_all[d0:d0 + cnt] = v[s0:s0 + cnt]
        idx_w = np.ascontiguousarray(
            np.tile(idx_all.reshape(TOT // 16, 16).T, (8, 1)))
        r_w = np.ascontiguousarray(r_all.reshape(TOT // 128, 128).T)
        v_w = np.ascontiguousarray(v_all.reshape(TOT // 128, 128).T)
        per_core.append({"idx16": idx_w, "rarr": r_w, "varr": v_w})
    return meta, per_core


def _build_program(cfg, meta, bias_mode):
    import concourse.bacc as bacc
    import concourse.mybir as mybir
    import concourse.tile as tile

    dt = mybir.dt
    f32 = dt.float32
    NCH, CW, BLK, NB, D = cfg.NCHUNK, cfg.CW, cfg.BLK, cfg.NB, cfg.D
    NSP = NB * BLK
    TOT = meta["TOT"]

    nc = bacc.Bacc("TRN2", target_bir_lowering=False, debug=False,
                   num_devices=cfg.NCORES)

    x_d = nc.dram_tensor("x", [cfg.N, D], f32, kind="ExternalInput")
    idx_d = nc.dram_tensor("idx16", [128, TOT // 16], dt.int16,
                           kind="ExternalInput")
    r_d = nc.dram_tensor("rarr", [128, TOT // 128], f32, kind="ExternalInput")
    v_d = nc.dram_tensor("varr", [128, TOT // 128], f32, kind="ExternalInput")
    w_d = nc.dram_tensor("w", [D, D], f32, kind="ExternalInput")
    iota_d = nc.dram_tensor("iota", [128, 128], f32, kind="ExternalInput")
    ident_d = nc.dram_tensor("ident", [128, 128], f32, kind="ExternalInput")
    if bias_mode:
        deg_d = nc.dram_tensor("deg", [1, NSP], f32, kind="ExternalInput")
        bt_d = nc.dram_tensor("bt", [D, 1], f32, kind="ExternalInput")
    out_d = nc.dram_tensor("out", [NSP, D], f32, kind="ExternalOutput")

    Copy = mybir.ActivationFunctionType.Copy
    Relu = mybir.ActivationFunctionType.Relu
    EQ = mybir.AluOpType.is_equal
    MUL = mybir.AluOpType.mult

    with tile.TileContext(nc) as tc:
        with (
            tc.tile_pool(name="const", bufs=1) as cpool,
            tc.tile_pool(name="gather", bufs=2) as gpool,
            tc.tile_pool(name="ptile", bufs=6) as ppool,
            tc.tile_pool(name="epi", bufs=3) as epool,
            tc.tile_pool(name="acc", bufs=2, space="PSUM") as acc_pool,
            tc.tile_pool(name="tps", bufs=2, space="PSUM") as tps_pool,
        ):
            sidx = cpool.tile([128, TOT // 16], dt.int16, tag="sidx")
            sr = cpool.tile([128, TOT // 128], f32, tag="sr")
            sv = cpool.tile([128, TOT // 128], f32, tag="sv")
            sw = cpool.tile([D, D], f32, tag="sw")
            siota = cpool.tile([128, 128], f32, tag="siota")
            sident = cpool.tile([128, 128], f32, tag="sident")
            nc.sync.dma_start(sidx[:], idx_d[:])
            nc.sync.dma_start(sr[:], r_d[:])
            nc.sync.dma_start(sv[:], v_d[:])
            nc.sync.dma_start(sw[:], w_d[:])
            nc.sync.dma_start(siota[:], iota_d[:])
            nc.sync.dma_start(sident[:], ident_d[:])
            if bias_mode:
                sdeg = cpool.tile([1, NSP], f32, tag="sdeg")
                sbt = cpool.tile([D, 1], f32, tag="sbt")
                nc.sync.dma_start(sdeg[:], deg_d[:])
                nc.sync.dma_start(sbt[:], bt_d[:])

            for sb in meta["sb_meta"]:
                gtiles = {}
                for c in range(NCH):
                    slots, off = sb["chunks"][c]
                    if slots == 0:
                        continue
                    g = gpool.tile([128, slots // 128, cfg.D], f32,
                                   tag=f"g{c}")
                    cap = getattr(cfg, "MAX_GATHER", 1 << 30)
                    for p0 in range(0, slots, cap):
                        n = min(cap, slots - p0)
                        nc.gpsimd.dma_gather(
                            g[:, p0 // 128:(p0 + n) // 128, :],
                            x_d[c * CW:(c + 1) * CW, :],
                            sidx[:, (off + p0) // 16:(off + p0 + n) // 16],
                            n,
                            n,
                            cfg.D,
                        )
                    gtiles[c] = g
                for b in sb["blocks"]:
                    seq = meta["blk_seq"][b]
                    if not seq:
                        continue
                    ps = acc_pool.tile([BLK, D], f32, tag="ps")
                    for i, (c, col, gt) in enumerate(seq):
                        P = ppool.tile([128, 128], f32, tag="P")
                        nc.vector.tensor_scalar(
                            P[:], siota[:], sr[:, gt:gt + 1],
                            sv[:, gt:gt + 1], EQ, MUL)
                        nc.tensor.matmul(
                            ps[:], P[:], gtiles[c][:, col, :],
                            start=(i == 0), stop=(i == len(seq) - 1))
                    # epilogue: out_b = relu(agg @ W (+ deg*b))
                    s1 = epool.tile([BLK, D], f32, tag="s1")
                    nc.scalar.activation(s1[:], ps[:], Copy)
                    p2 = tps_pool.tile([D, BLK], f32, tag="p2")
                    nc.tensor.transpose(p2[:], s1[:], sident[:])
                    s2 = epool.tile([D, BLK], f32, tag="s2")
                    nc.scalar.activation(s2[:], p2[:], Copy)
                    p3 = tps_pool.tile([D, BLK], f32, tag="p3")
                    nc.tensor.matmul(p3[:], sw[:], s2[:],
                                     start=True, stop=True)
                    s3 = epool.tile([D, BLK], f32, tag="s3")
                    if bias_mode:
                        sba = epool.tile([D, BLK], f32, tag="sba")
                        nc.vector.tensor_scalar(
                            sba[:],
                            sdeg[0:1, b * BLK:(b + 1) * BLK]
                            .to_broadcast([D, BLK]),
                            sbt[:], None, MUL)
                        nc.vector.tensor_tensor(
                            s3[:], p3[:], sba[:], mybir.AluOpType.add)
                        nc.scalar.activation(s3[:], s3[:], Relu)
                    else:
                        nc.scalar.activation(s3[:], p3[:], Relu)
                    p4 = acc_pool.tile([BLK, D], f32, tag="p4")
                    nc.tensor.transpose(p4[:], s3[:], sident[:D, :D])
                    s4 = epool.tile([BLK, D], f32, tag="s4")
                    nc.scalar.activation(s4[:], p4[:], Copy)
                    nc.sync.dma_start(out_d[b * BLK:(b + 1) * BLK, :], s4[:])

    nc.compile()
    return nc


_CACHE = {}


def _get_program(cfg, meta, bias_mode):
    key = (id(cfg), meta["TOT"], bias_mode,
           tuple(meta["Q"].ravel().tolist()))
    if key not in _CACHE:
        _CACHE[key] = _build_program(cfg, meta, bias_mode)
    return _CACHE[key]


def kernel(x, adj_vals, W, b, edge_src, edge_dst, _cfg=None, _timing=None):
    from concourse.bass_utils import run_bass_kernel_spmd

    cfg = _cfg or CFG
    x = np.ascontiguousarray(np.asarray(x, np.float32))
    adj_vals = np.asarray(adj_vals, np.float32)
    W = np.ascontiguousarray(np.asarray(W, np.float32))
    b = np.asarray(b, np.float32)
    edge_src = np.asarray(edge_src, np.int64)
    edge_dst = np.asarray(edge_dst, np.int64)

    bias_mode = bool(np.any(b != 0))
    meta, per_core = _prepare(cfg, adj_vals, edge_src, edge_dst)
    nc = _get_program(cfg, meta, bias_mode)

    iota = np.tile(np.arange(128, dtype=np.float32), (128, 1))
    ident = np.eye(128, dtype=np.float32)
    NSP = cfg.NB * cfg.BLK

    in_maps = []
    for m in range(cfg.NCORES):
        im = {
            "x": x,
            "idx16": per_core[m]["idx16"],
            "rarr": per_core[m]["rarr"],
            "varr": per_core[m]["varr"],
            "w": W,
            "iota": iota,
            "ident": ident,
        }
        if bias_mode:
            deg = np.zeros(NSP, np.float32)
            sel = edge_dst // cfg.NS == m
            np.add.at(deg, (edge_dst[sel] - m * cfg.NS), adj_vals[sel])
            im["deg"] = deg[None, :]
            im["bt"] = b.reshape(cfg.D, 1)
        in_maps.append(im)

    res = run_bass_kernel_spmd(nc, in_maps, core_ids=list(range(cfg.NCORES)))
    if _timing is not None:
        for _ in range(_timing.get("iters", 2)):
            t0 = time.time()
            res = run_bass_kernel_spmd(
                nc, in_maps, core_ids=list(range(cfg.NCORES)))
            _timing.setdefault("wall_s", []).append(time.time() - t0)

    out = np.empty((cfg.N, cfg.D), np.float32)
    for m in range(cfg.NCORES):
        out[m * cfg.NS:(m + 1) * cfg.NS] = res.results[m]["out"][:cfg.NS]
    return out


# revision 5
# speedup vs baseline: 36.2674x; 36.2674x over previous
"""Trainium2 Bass kernel for BatchGraphConv (GNN message passing).

out = relu(segment_sum(adj_vals * (x@W+b)[edge_src], edge_dst))
    = relu(agg @ W + deg * b)   where agg[i] = sum_e v_e x[src_e], deg[i] = sum_e v_e

Sharding: destination nodes split across 8 cores (12500 each). Each core:
  - hardware dma_gather of x rows for its edges (grouped by 128-node dst
    block and 25000-row src chunk so indices fit int16)
  - per 128-edge tile: P = (iota == r) * v  (value-weighted one-hot, DVE)
  - TensorE: psum_block += P^T @ G  (segment sum into 128-node block)
  - epilogue per block: transpose, @W, (+deg*b), relu, transpose, DMA out.
Host does index bookkeeping only (sort/group/pad); all FLOPs on device.
"""

import os
import sys
import time

import numpy as np

for _p in ("/opt/trn_rl_repo", "/root/.axon_site/_ro/trn_rl_repo"):
    if os.path.isdir(_p) and _p not in sys.path:
        sys.path.insert(0, _p)


class CFG:
    N = 100000
    E = 1600000
    D = 64
    NCORES = 8
    NS = 12500          # dst nodes per core
    BLK = 128           # max nodes per block (= PSUM partitions)
    NB = 98             # fixed-block count (v1 path)
    NCHUNK = 4          # src index windows
    CW = 25000          # src chunk width (int16-addressable rows)
    SB_BLOCKS = 8       # blocks per superblock (gather batch)
    MAX_GATHER = 1024   # max indices per dma_gather instruction (HW limit)
    QSLOTS = 512        # slots per (block, chunk); multiple of 128
    P_ACT_EVERY = 0     # 0=off; else every k-th P-build goes to ScalarE


def _ceil_to(a, m):
    return -(-a // m) * m


def _prepare(cfg, adj_vals, edge_src, edge_dst):
    """Host-side index prep. Returns (meta, per_core_arrays)."""
    NC, NS, BLK, NB, NCH, CW = (
        cfg.NCORES, cfg.NS, cfg.BLK, cfg.NB, cfg.NCHUNK, cfg.CW)
    nreg = NB * NCH

    core_of = edge_dst // NS
    per_core_raw = []
    counts = np.zeros((NC, NB, NCH), np.int64)
    for m in range(NC):
        sel = np.nonzero(core_of == m)[0]
        ldst = edge_dst[sel] - m * NS
        blk = ldst // BLK
        r = (ldst % BLK).astype(np.float32)
        ch = edge_src[sel] // CW
        srcrel = (edge_src[sel] - ch * CW).astype(np.int16)
        key = blk * NCH + ch
        order = np.argsort(key, kind="stable")
        key = key[order]
        starts = np.searchsorted(key, np.arange(nreg + 1))
        counts[m] = np.diff(starts).reshape(NB, NCH)
        per_core_raw.append((srcrel[order], r[order],
                             adj_vals[sel[order]].astype(np.float32), starts))

    # padded region sizes, shared across cores
    Q = np.array([[_ceil_to(int(c), 128) if c else 0
                   for c in row] for row in counts.max(axis=0)], np.int64)
    # ensure every block has at least one tile (so psum is initialized)
    for b in range(NB):
        if Q[b].sum() == 0:
            Q[b, 0] = 128

    # superblocks
    sb_list = [list(range(s, min(s + cfg.SB_BLOCKS, NB)))
               for s in range(0, NB, cfg.SB_BLOCKS)]

    # global slot layout: for sb -> for c -> for b in sb
    slot_off = 0
    regions = {}          # (b, c) -> global slot offset
    sb_meta = []          # per sb: {c: (slots, off_slots)}
    for blocks in sb_list:
        cmeta = {}
        for c in range(NCH):
            off_c = slot_off
            for b in blocks:
                regions[(b, c)] = slot_off
                slot_off += int(Q[b, c])
            cmeta[c] = (slot_off - off_c, off_c)
        sb_meta.append({"blocks": blocks, "chunks": cmeta})
    TOT = slot_off
    assert TOT % 128 == 0

    # per-block matmul sequence: (c, tile col within (sb,c) buffer, global tile)
    blk_seq = [[] for _ in range(NB)]
    for sbi, blocks in enumerate(sb_list):
        for c in range(NCH):
            _, off_c = sb_meta[sbi]["chunks"][c]
            for b in blocks:
                roff = regions[(b, c)]
                for t in range(int(Q[b, c]) // 128):
                    blk_seq[b].append(
                        (c, (roff - off_c) // 128 + t, roff // 128 + t))

    meta = {"Q": Q, "sb_meta": sb_meta, "blk_seq": blk_seq, "TOT": TOT}

    # per-core slot arrays
    per_core = []
    for m in range(NC):
        srcrel, r, v, starts = per_core_raw[m]
        idx_all = np.zeros(TOT, np.int16)
        r_all = np.zeros(TOT, np.float32)
        v_all = np.zeros(TOT, np.float32)
        for b in range(NB):
            for c in range(NCH):
                cnt = int(counts[m, b, c])
                if cnt == 0:
                    continue
                s0 = starts[b * NCH + c]
                d0 = regions[(b, c)]
                idx_all[d0:d0 + cnt] = srcrel[s0:s0 + cnt]
                r_all[d0:d0 + cnt] = r[s0:s0 + cnt]
                v_all[d0:d0 + cnt] = v[s0:s0 + cnt]
        idx_w = np.ascontiguousarray(
            np.tile(idx_all.reshape(TOT // 16, 16).T, (8, 1)))
        r_w = np.ascontiguousarray(r_all.reshape(TOT // 128, 128).T)
        v_w = np.ascontiguousarray(v_all.reshape(TOT // 128, 128).T)
        per_core.append({"idx16": idx_w, "rarr": r_w, "varr": v_w})
    return meta, per_core


def _build_program(cfg, meta, bias_mode):
    import concourse.bacc as bacc
    import concourse.mybir as mybir
    import concourse.tile as tile

    dt = mybir.dt
    f32 = dt.float32
    NCH, CW, BLK, NB, D = cfg.NCHUNK, cfg.CW, cfg.BLK, cfg.NB, cfg.D
    NSP = NB * BLK
    TOT = meta["TOT"]

    nc = bacc.Bacc("TRN2", target_bir_lowering=False, debug=False,
                   num_devices=cfg.NCORES)

    x_d = nc.dram_tensor("x", [cfg.N, D], f32, kind="ExternalInput")
    idx_d = nc.dram_tensor("idx16", [128, TOT // 16], dt.int16,
                           kind="ExternalInput")
    r_d = nc.dram_tensor("rarr", [128, TOT // 128], f32, kind="ExternalInput")
    v_d = nc.dram_tensor("varr", [128, TOT // 128], f32, kind="ExternalInput")
    w_d = nc.dram_tensor("w", [D, D], f32, kind="ExternalInput")
    iota_d = nc.dram_tensor("iota", [128, 128], f32, kind="ExternalInput")
    ident_d = nc.dram_tensor("ident", [128, 128], f32, kind="ExternalInput")
    if bias_mode:
        deg_d = nc.dram_tensor("deg", [1, NSP], f32, kind="ExternalInput")
        bt_d = nc.dram_tensor("bt", [D, 1], f32, kind="ExternalInput")
    out_d = nc.dram_tensor("out", [NSP, D], f32, kind="ExternalOutput")

    Copy = mybir.ActivationFunctionType.Copy
    Relu = mybir.ActivationFunctionType.Relu
    EQ = mybir.AluOpType.is_equal
    MUL = mybir.AluOpType.mult

    with tile.TileContext(nc) as tc:
        with (
            tc.tile_pool(name="const", bufs=1) as cpool,
            tc.tile_pool(name="gather", bufs=2) as gpool,
            tc.tile_pool(name="ptile", bufs=6) as ppool,
            tc.tile_pool(name="epi", bufs=3) as epool,
            tc.tile_pool(name="acc", bufs=2, space="PSUM") as acc_pool,
            tc.tile_pool(name="tps", bufs=2, space="PSUM") as tps_pool,
        ):
            sidx = cpool.tile([128, TOT // 16], dt.int16, tag="sidx")
            sr = cpool.tile([128, TOT // 128], f32, tag="sr")
            sv = cpool.tile([128, TOT // 128], f32, tag="sv")
            sw = cpool.tile([D, D], f32, tag="sw")
            siota = cpool.tile([128, 128], f32, tag="siota")
            sident = cpool.tile([128, 128], f32, tag="sident")
            nc.sync.dma_start(sidx[:], idx_d[:])
            nc.sync.dma_start(sr[:], r_d[:])
            nc.sync.dma_start(sv[:], v_d[:])
            nc.sync.dma_start(sw[:], w_d[:])
            nc.sync.dma_start(siota[:], iota_d[:])
            nc.sync.dma_start(sident[:], ident_d[:])
            if bias_mode:
                sdeg = cpool.tile([1, NSP], f32, tag="sdeg")
                sbt = cpool.tile([D, 1], f32, tag="sbt")
                nc.sync.dma_start(sdeg[:], deg_d[:])
                nc.sync.dma_start(sbt[:], bt_d[:])

            for sb in meta["sb_meta"]:
                gtiles = {}
                for c in range(NCH):
                    slots, off = sb["chunks"][c]
                    if slots == 0:
                        continue
                    g = gpool.tile([128, slots // 128, cfg.D], f32,
                                   tag=f"g{c}")
                    cap = getattr(cfg, "MAX_GATHER", 1 << 30)
                    for p0 in range(0, slots, cap):
                        n = min(cap, slots - p0)
                        nc.gpsimd.dma_gather(
                            g[:, p0 // 128:(p0 + n) // 128, :],
                            x_d[c * CW:(c + 1) * CW, :],
                            sidx[:, (off + p0) // 16:(off + p0 + n) // 16],
                            n,
                            n,
                            cfg.D,
                        )
                    gtiles[c] = g
                for b in sb["blocks"]:
                    seq = meta["blk_seq"][b]
                    if not seq:
                        continue
                    ps = acc_pool.tile([BLK, D], f32, tag="ps")
                    for i, (c, col, gt) in enumerate(seq):
                        P = ppool.tile([128, 128], f32, tag="P")
                        nc.vector.tensor_scalar(
                            P[:], siota[:], sr[:, gt:gt + 1],
                            sv[:, gt:gt + 1], EQ, MUL)
                        nc.tensor.matmul(
                            ps[:], P[:], gtiles[c][:, col, :],
                            start=(i == 0), stop=(i == len(seq) - 1))
                    # epilogue: out_b = relu(agg @ W (+ deg*b))
                    s1 = epool.tile([BLK, D], f32, tag="s1")
                    nc.scalar.activation(s1[:], ps[:], Copy)
                    p2 = tps_pool.tile([D, BLK], f32, tag="p2")
                    nc.tensor.transpose(p2[:], s1[:], sident[:])
                    s2 = epool.tile([D, BLK], f32, tag="s2")
                    nc.scalar.activation(s2[:], p2[:], Copy)
                    p3 = tps_pool.tile([D, BLK], f32, tag="p3")
                    nc.tensor.matmul(p3[:], sw[:], s2[:],
                                     start=True, stop=True)
                    s3 = epool.tile([D, BLK], f32, tag="s3")
                    if bias_mode:
                        sba = epool.tile([D, BLK], f32, tag="sba")
                        nc.vector.tensor_scalar(
                            sba[:],
                            sdeg[0:1, b * BLK:(b + 1) * BLK]
                            .to_broadcast([D, BLK]),
                            sbt[:], None, MUL)
                        nc.vector.tensor_tensor(
                            s3[:], p3[:], sba[:], mybir.AluOpType.add)
                        nc.scalar.activation(s3[:], s3[:], Relu)
                    else:
                        nc.scalar.activation(s3[:], p3[:], Relu)
                    p4 = acc_pool.tile([BLK, D], f32, tag="p4")
                    nc.tensor.transpose(p4[:], s3[:], sident[:D, :D])
                    s4 = epool.tile([BLK, D], f32, tag="s4")
                    nc.scalar.activation(s4[:], p4[:], Copy)
                    nc.sync.dma_start(out_d[b * BLK:(b + 1) * BLK, :], s4[:])

    nc.compile()
    return nc


_CACHE = {}


def _get_program(cfg, meta, bias_mode):
    key = (id(cfg), meta["TOT"], bias_mode,
           tuple(meta["Q"].ravel().tolist()))
    if key not in _CACHE:
        _CACHE[key] = _build_program(cfg, meta, bias_mode)
    return _CACHE[key]


def kernel(x, adj_vals, W, b, edge_src, edge_dst, _cfg=None, _timing=None):
    from concourse.bass_utils import run_bass_kernel_spmd

    cfg = _cfg or CFG
    x = np.ascontiguousarray(np.asarray(x, np.float32))
    adj_vals = np.asarray(adj_vals, np.float32)
    W = np.ascontiguousarray(np.asarray(W, np.float32))
    b = np.asarray(b, np.float32)
    edge_src = np.asarray(edge_src, np.int64)
    edge_dst = np.asarray(edge_dst, np.int64)

    bias_mode = bool(np.any(b != 0))
    meta, per_core = _prepare(cfg, adj_vals, edge_src, edge_dst)
    nc = _get_program(cfg, meta, bias_mode)

    iota = np.tile(np.arange(128, dtype=np.float32), (128, 1))
    ident = np.eye(128, dtype=np.float32)
    NSP = cfg.NB * cfg.BLK

    in_maps = []
    for m in range(cfg.NCORES):
        im = {
            "x": x,
            "idx16": per_core[m]["idx16"],
            "rarr": per_core[m]["rarr"],
            "varr": per_core[m]["varr"],
            "w": W,
            "iota": iota,
            "ident": ident,
        }
        if bias_mode:
            deg = np.zeros(NSP, np.float32)
            sel = edge_dst // cfg.NS == m
            np.add.at(deg, (edge_dst[sel] - m * cfg.NS), adj_vals[sel])
            im["deg"] = deg[None, :]
            im["bt"] = b.reshape(cfg.D, 1)
        in_maps.append(im)

    res = run_bass_kernel_spmd(nc, in_maps, core_ids=list(range(cfg.NCORES)))
    if _timing is not None:
        for _ in range(_timing.get("iters", 2)):
            t0 = time.time()
            res = run_bass_kernel_spmd(
                nc, in_maps, core_ids=list(range(cfg.NCORES)))
            _timing.setdefault("wall_s", []).append(time.time() - t0)

    out = np.empty((cfg.N, cfg.D), np.float32)
    for m in range(cfg.NCORES):
        out[m * cfg.NS:(m + 1) * cfg.NS] = res.results[m]["out"][:cfg.NS]
    return out


# revision 8
# speedup vs baseline: 49.7461x; 1.3716x over previous
"""Trainium2 Bass kernel for BatchGraphConv (GNN message passing).

out = relu(segment_sum(adj_vals * (x@W+b)[edge_src], edge_dst))
    = relu(agg @ W + deg * b)   where agg[i] = sum_e v_e x[src_e], deg[i] = sum_e v_e

Sharding: destination nodes split across 8 cores (12500 each). Each core:
  - hardware dma_gather of x rows for its edges (grouped by 128-node dst
    block and 25000-row src chunk so indices fit int16)
  - per 128-edge tile: P = (iota == r) * v  (value-weighted one-hot, DVE)
  - TensorE: psum_block += P^T @ G  (segment sum into 128-node block)
  - epilogue per block: transpose, @W, (+deg*b), relu, transpose, DMA out.
Host does index bookkeeping only (sort/group/pad); all FLOPs on device.
"""

import os
import sys
import time

import numpy as np

for _p in ("/opt/trn_rl_repo", "/root/.axon_site/_ro/trn_rl_repo"):
    if os.path.isdir(_p) and _p not in sys.path:
        sys.path.insert(0, _p)


class CFG:
    N = 100000
    E = 1600000
    D = 64
    NCORES = 8
    NS = 12500          # dst nodes per core
    BLK = 128           # max nodes per block (= PSUM partitions)
    NB = 98             # fixed-block count (v1 path)
    NCHUNK = 4          # src index windows
    CW = 25000          # src chunk width (int16-addressable rows)
    SB_BLOCKS = 8       # blocks per superblock (gather batch)
    MAX_GATHER = 1024   # max indices per dma_gather instruction (HW limit)
    QSLOTS = 512        # slots per (block, chunk); multiple of 128
    P_ACT_EVERY = 0     # 0=off; else every k-th P-build goes to ScalarE


def _ceil_to(a, m):
    return -(-a // m) * m


def _prepare(cfg, adj_vals, edge_src, edge_dst):
    """Host-side index prep. Returns (meta, per_core_arrays)."""
    NC, NS, BLK, NB, NCH, CW = (
        cfg.NCORES, cfg.NS, cfg.BLK, cfg.NB, cfg.NCHUNK, cfg.CW)
    nreg = NB * NCH

    core_of = edge_dst // NS
    per_core_raw = []
    counts = np.zeros((NC, NB, NCH), np.int64)
    for m in range(NC):
        sel = np.nonzero(core_of == m)[0]
        ldst = edge_dst[sel] - m * NS
        blk = ldst // BLK
        r = (ldst % BLK).astype(np.float32)
        ch = edge_src[sel] // CW
        srcrel = (edge_src[sel] - ch * CW).astype(np.int16)
        key = blk * NCH + ch
        order = np.argsort(key, kind="stable")
        key = key[order]
        starts = np.searchsorted(key, np.arange(nreg + 1))
        counts[m] = np.diff(starts).reshape(NB, NCH)
        per_core_raw.append((srcrel[order], r[order],
                             adj_vals[sel[order]].astype(np.float32), starts))

    # padded region sizes, shared across cores
    Q = np.array([[_ceil_to(int(c), 128) if c else 0
                   for c in row] for row in counts.max(axis=0)], np.int64)
    # ensure every block has at least one tile (so psum is initialized)
    for b in range(NB):
        if Q[b].sum() == 0:
            Q[b, 0] = 128

    # superblocks
    sb_list = [list(range(s, min(s + cfg.SB_BLOCKS, NB)))
               for s in range(0, NB, cfg.SB_BLOCKS)]

    # global slot layout: for sb -> for c -> for b in sb
    slot_off = 0
    regions = {}          # (b, c) -> global slot offset
    sb_meta = []          # per sb: {c: (slots, off_slots)}
    for blocks in sb_list:
        cmeta = {}
        for c in range(NCH):
            off_c = slot_off
            for b in blocks:
                regions[(b, c)] = slot_off
                slot_off += int(Q[b, c])
            cmeta[c] = (slot_off - off_c, off_c)
        sb_meta.append({"blocks": blocks, "chunks": cmeta})
    TOT = slot_off
    assert TOT % 128 == 0

    # per-block matmul sequence: (c, tile col within (sb,c) buffer, global tile)
    blk_seq = [[] for _ in range(NB)]
    for sbi, blocks in enumerate(sb_list):
        for c in range(NCH):
            _, off_c = sb_meta[sbi]["chunks"][c]
            for b in blocks:
                roff = regions[(b, c)]
                for t in range(int(Q[b, c]) // 128):
                    blk_seq[b].append(
                        (c, (roff - off_c) // 128 + t, roff // 128 + t))

    meta = {"Q": Q, "sb_meta": sb_meta, "blk_seq": blk_seq, "TOT": TOT}

    # per-core slot arrays
    per_core = []
    for m in range(NC):
        srcrel, r, v, starts = per_core_raw[m]
        idx_all = np.zeros(TOT, np.int16)
        r_all = np.zeros(TOT, np.float32)
        v_all = np.zeros(TOT, np.float32)
        for b in range(NB):
            for c in range(NCH):
                cnt = int(counts[m, b, c])
                if cnt == 0:
                    continue
                s0 = starts[b * NCH + c]
                d0 = regions[(b, c)]
                idx_all[d0:d0 + cnt] = srcrel[s0:s0 + cnt]
                r_all[d0:d0 + cnt] = r[s0:s0 + cnt]
                v_all[d0:d0 + cnt] = v[s0:s0 + cnt]
        idx_w = np.ascontiguousarray(
            np.tile(idx_all.reshape(TOT // 16, 16).T, (8, 1)))
        r_w = np.ascontiguousarray(r_all.reshape(TOT // 128, 128).T)
        v_w = np.ascontiguousarray(v_all.reshape(TOT // 128, 128).T)
        per_core.append({"idx16": idx_w, "rarr": r_w, "varr": v_w})
    return meta, per_core


def _build_program(cfg, meta, bias_mode):
    import concourse.bacc as bacc
    import concourse.mybir as mybir
    import concourse.tile as tile

    dt = mybir.dt
    f32 = dt.float32
    NCH, CW, BLK, NB, D = cfg.NCHUNK, cfg.CW, cfg.BLK, cfg.NB, cfg.D
    NSP = NB * BLK
    TOT = meta["TOT"]

    nc = bacc.Bacc("TRN2", target_bir_lowering=False, debug=False,
                   num_devices=cfg.NCORES,
                   num_swdge_queues=getattr(cfg, "SWDGE_QUEUES", 1))

    x_d = nc.dram_tensor("x", [cfg.N, D], f32, kind="ExternalInput")
    idx_d = nc.dram_tensor("idx16", [128, TOT // 16], dt.int16,
                           kind="ExternalInput")
    r_d = nc.dram_tensor("rarr", [128, TOT // 128], f32, kind="ExternalInput")
    v_d = nc.dram_tensor("varr", [128, TOT // 128], f32, kind="ExternalInput")
    w_d = nc.dram_tensor("w", [D, D], f32, kind="ExternalInput")
    iota_d = nc.dram_tensor("iota", [128, 128], f32, kind="ExternalInput")
    ident_d = nc.dram_tensor("ident", [128, 128], f32, kind="ExternalInput")
    if bias_mode:
        deg_d = nc.dram_tensor("deg", [1, NSP], f32, kind="ExternalInput")
        bt_d = nc.dram_tensor("bt", [D, 1], f32, kind="ExternalInput")
    out_d = nc.dram_tensor("out", [NSP, D], f32, kind="ExternalOutput")

    Copy = mybir.ActivationFunctionType.Copy
    Relu = mybir.ActivationFunctionType.Relu
    EQ = mybir.AluOpType.is_equal
    MUL = mybir.AluOpType.mult

    with tile.TileContext(nc) as tc:
        with (
            tc.tile_pool(name="const", bufs=1) as cpool,
            tc.tile_pool(name="gather", bufs=2) as gpool,
            tc.tile_pool(name="ptile", bufs=6) as ppool,
            tc.tile_pool(name="epi", bufs=3) as epool,
            tc.tile_pool(name="acc", bufs=2, space="PSUM") as acc_pool,
            tc.tile_pool(name="tps", bufs=2, space="PSUM") as tps_pool,
        ):
            sidx = cpool.tile([128, TOT // 16], dt.int16, tag="sidx")
            sr = cpool.tile([128, TOT // 128], f32, tag="sr")
            sv = cpool.tile([128, TOT // 128], f32, tag="sv")
            sw = cpool.tile([D, D], f32, tag="sw")
            siota = cpool.tile([128, 128], f32, tag="siota")
            sident = cpool.tile([128, 128], f32, tag="sident")
            nc.sync.dma_start(sidx[:], idx_d[:])
            nc.sync.dma_start(sr[:], r_d[:])
            nc.sync.dma_start(sv[:], v_d[:])
            nc.sync.dma_start(sw[:], w_d[:])
            nc.sync.dma_start(siota[:], iota_d[:])
            nc.sync.dma_start(sident[:], ident_d[:])
            if bias_mode:
                sdeg = cpool.tile([1, NSP], f32, tag="sdeg")
                sbt = cpool.tile([D, 1], f32, tag="sbt")
                nc.sync.dma_start(sdeg[:], deg_d[:])
                nc.sync.dma_start(sbt[:], bt_d[:])

            for sb in meta["sb_meta"]:
                gtiles = {}
                for c in range(NCH):
                    slots, off = sb["chunks"][c]
                    if slots == 0:
                        continue
                    g = gpool.tile([128, slots // 128, cfg.D], f32,
                                   tag=f"g{c}")
                    cap = getattr(cfg, "MAX_GATHER", 1 << 30)
                    nq = getattr(cfg, "SWDGE_QUEUES", 1)
                    sp = bool(getattr(cfg, "SINGLE_PACKET", True))
                    for p0 in range(0, slots, cap):
                        n = min(cap, slots - p0)
                        nc.gpsimd.dma_gather(
                            g[:, p0 // 128:(p0 + n) // 128, :],
                            x_d[c * CW:(c + 1) * CW, :],
                            sidx[:, (off + p0) // 16:(off + p0 + n) // 16],
                            n,
                            n,
                            cfg.D,
                            single_packet=sp,
                            queue_num=(c % nq),
                        )
                    gtiles[c] = g
                for b in sb["blocks"]:
                    seq = meta["blk_seq"][b]
                    if not seq:
                        continue
                    ps = acc_pool.tile([BLK, D], f32, tag="ps")
                    for i, (c, col, gt) in enumerate(seq):
                        P = ppool.tile([128, 128], f32, tag="P")
                        nc.vector.tensor_scalar(
                            P[:], siota[:], sr[:, gt:gt + 1],
                            sv[:, gt:gt + 1], EQ, MUL)
                        nc.tensor.matmul(
                            ps[:], P[:], gtiles[c][:, col, :],
                            start=(i == 0), stop=(i == len(seq) - 1))
                    # epilogue: out_b = relu(agg @ W (+ deg*b))
                    s1 = epool.tile([BLK, D], f32, tag="s1")
                    nc.scalar.activation(s1[:], ps[:], Copy)
                    p2 = tps_pool.tile([D, BLK], f32, tag="p2")
                    nc.tensor.transpose(p2[:], s1[:], sident[:])
                    s2 = epool.tile([D, BLK], f32, tag="s2")
                    nc.scalar.activation(s2[:], p2[:], Copy)
                    p3 = tps_pool.tile([D, BLK], f32, tag="p3")
                    nc.tensor.matmul(p3[:], sw[:], s2[:],
                                     start=True, stop=True)
                    s3 = epool.tile([D, BLK], f32, tag="s3")
                    if bias_mode:
                        sba = epool.tile([D, BLK], f32, tag="sba")
                        nc.vector.tensor_scalar(
                            sba[:],
                            sdeg[0:1, b * BLK:(b + 1) * BLK]
                            .to_broadcast([D, BLK]),
                            sbt[:], None, MUL)
                        nc.vector.tensor_tensor(
                            s3[:], p3[:], sba[:], mybir.AluOpType.add)
                        nc.scalar.activation(s3[:], s3[:], Relu)
                    else:
                        nc.scalar.activation(s3[:], p3[:], Relu)
                    p4 = acc_pool.tile([BLK, D], f32, tag="p4")
                    nc.tensor.transpose(p4[:], s3[:], sident[:D, :D])
                    s4 = epool.tile([BLK, D], f32, tag="s4")
                    nc.scalar.activation(s4[:], p4[:], Copy)
                    nc.sync.dma_start(out_d[b * BLK:(b + 1) * BLK, :], s4[:])

    nc.compile()
    return nc


_CACHE = {}


def _get_program(cfg, meta, bias_mode):
    key = (id(cfg), meta["TOT"], bias_mode,
           tuple(meta["Q"].ravel().tolist()))
    if key not in _CACHE:
        _CACHE[key] = _build_program(cfg, meta, bias_mode)
    return _CACHE[key]


def kernel(x, adj_vals, W, b, edge_src, edge_dst, _cfg=None, _timing=None):
    from concourse.bass_utils import run_bass_kernel_spmd

    cfg = _cfg or CFG
    x = np.ascontiguousarray(np.asarray(x, np.float32))
    adj_vals = np.asarray(adj_vals, np.float32)
    W = np.ascontiguousarray(np.asarray(W, np.float32))
    b = np.asarray(b, np.float32)
    edge_src = np.asarray(edge_src, np.int64)
    edge_dst = np.asarray(edge_dst, np.int64)

    bias_mode = bool(np.any(b != 0))
    meta, per_core = _prepare(cfg, adj_vals, edge_src, edge_dst)
    nc = _get_program(cfg, meta, bias_mode)

    iota = np.tile(np.arange(128, dtype=np.float32), (128, 1))
    ident = np.eye(128, dtype=np.float32)
    NSP = cfg.NB * cfg.BLK

    in_maps = []
    for m in range(cfg.NCORES):
        im = {
            "x": x,
            "idx16": per_core[m]["idx16"],
            "rarr": per_core[m]["rarr"],
            "varr": per_core[m]["varr"],
            "w": W,
            "iota": iota,
            "ident": ident,
        }
        if bias_mode:
            deg = np.zeros(NSP, np.float32)
            sel = edge_dst // cfg.NS == m
            np.add.at(deg, (edge_dst[sel] - m * cfg.NS), adj_vals[sel])
            im["deg"] = deg[None, :]
            im["bt"] = b.reshape(cfg.D, 1)
        in_maps.append(im)

    res = run_bass_kernel_spmd(nc, in_maps, core_ids=list(range(cfg.NCORES)))
    if _timing is not None:
        for _ in range(_timing.get("iters", 2)):
            t0 = time.time()
            res = run_bass_kernel_spmd(
                nc, in_maps, core_ids=list(range(cfg.NCORES)))
            _timing.setdefault("wall_s", []).append(time.time() - t0)

    out = np.empty((cfg.N, cfg.D), np.float32)
    for m in range(cfg.NCORES):
        out[m * cfg.NS:(m + 1) * cfg.NS] = res.results[m]["out"][:cfg.NS]
    return out


# revision 22
# speedup vs baseline: 76.8587x; 1.5450x over previous
"""Trainium2 Bass kernel for BatchGraphConv (GNN message passing).

out = relu(segment_sum(adj_vals * (x@W+b)[edge_src], edge_dst))
    = relu(agg @ W + deg * b)   where agg[i] = sum_e v_e x[src_e], deg[i] = sum_e v_e

Sharding: destination nodes split across 8 cores (12500 each). Each core:
  - hardware dma_gather of x rows for its edges (grouped by 128-node dst
    block and 25000-row src chunk so indices fit int16)
  - per 128-edge tile: P = (iota == r) * v  (value-weighted one-hot, DVE)
  - TensorE: psum_block += P^T @ G  (segment sum into 128-node block)
  - epilogue per block: transpose, @W, (+deg*b), relu, transpose, DMA out.
Host does index bookkeeping only (sort/group/pad); all FLOPs on device.
"""

import os
import sys
import time

import numpy as np

for _p in ("/opt/trn_rl_repo", "/root/.axon_site/_ro/trn_rl_repo"):
    if os.path.isdir(_p) and _p not in sys.path:
        sys.path.insert(0, _p)


class CFG:
    N = 100000
    E = 1600000
    D = 64
    NCORES = 8
    NS = 12500          # dst nodes per core
    BLK = 128           # max nodes per block (= PSUM partitions)
    NB = 98             # fixed-block count (v1 path)
    NCHUNK = 4          # src index windows
    CW = 25000          # src chunk width (int16-addressable rows)
    SB_BLOCKS = 8       # blocks per superblock (gather batch)
    MAX_GATHER = 1024   # max indices per dma_gather instruction (HW limit)
    QSLOTS = 512        # slots per (block, chunk); multiple of 128
    P_ACT_EVERY = 0     # 0=off; else every k-th P-build goes to ScalarE


def _ceil_to(a, m):
    return -(-a // m) * m


def _prepare(cfg, adj_vals, edge_src, edge_dst):
    """Host-side index prep with variable-size dst blocks.

    Each block covers <=128 consecutive dst nodes, chosen per core so that
    its edge count per src-chunk fits a fixed budget Q=cfg.QSLOTS. Every
    block therefore has an identical device-side structure (NCHUNK regions
    of Q slots = Q/128 tiles each); only the data differs per core.
    Returns (meta, per_core) where per_core[m] has idx16/rarr/varr slot
    arrays plus rowmap (padded out-row of each real node).
    """
    NC, NS, BLK, NCH, CW, Q = (
        cfg.NCORES, cfg.NS, cfg.BLK, cfg.NCHUNK, cfg.CW, cfg.QSLOTS)
    assert Q % 128 == 0

    core_of = edge_dst // NS
    cores = []
    nblocks = []
    for m in range(NC):
        sel = np.nonzero(core_of == m)[0]
        ldst = edge_dst[sel] - m * NS
        ch = edge_src[sel] // CW
        # per-node per-chunk counts
        cnt = np.zeros((NS, NCH), np.int64)
        np.add.at(cnt, (ldst, ch), 1)
        assert (cnt <= Q).all(), "single node exceeds chunk budget"
        # greedy pack nodes into blocks
        bstart = [0]
        cur = np.zeros(NCH, np.int64)
        nodes = 0
        for n in range(NS):
            nxt = cur + cnt[n]
            if nodes == BLK or (nxt > Q).any():
                bstart.append(n)
                cur = cnt[n].copy()
                nodes = 1
            else:
                cur = nxt
                nodes += 1
        bstart = np.asarray(bstart + [NS])
        nblocks.append(len(bstart) - 1)
        # sort edges by (block, chunk)
        blk_of_node = np.zeros(NS, np.int64)
        blk_of_node[bstart[1:-1]] = 1
        blk_of_node = np.cumsum(blk_of_node)
        blk = blk_of_node[ldst]
        r = (ldst - bstart[blk]).astype(np.float32)
        srcrel = (edge_src[sel] - ch * CW).astype(np.int16)
        key = blk * NCH + ch
        order = np.argsort(key, kind="stable")
        nb = len(bstart) - 1
        starts = np.searchsorted(key[order], np.arange(nb * NCH + 1))
        cores.append({
            "bstart": bstart, "nb": nb, "starts": starts,
            "srcrel": srcrel[order], "r": r[order],
            "v": adj_vals[sel][order].astype(np.float32),
        })

    B = max(nblocks)
    # uniform layout: superblocks of SB_BLOCKS blocks; per (sb, c):
    # len(blocks)*Q slots, block regions in order.
    sb_list = [list(range(s, min(s + cfg.SB_BLOCKS, B)))
               for s in range(0, B, cfg.SB_BLOCKS)]
    slot_off = 0
    regions = {}
    sb_meta = []
    for blocks in sb_list:
        cmeta = {}
        for c in range(NCH):
            off_c = slot_off
            for b in blocks:
                regions[(b, c)] = slot_off
                slot_off += Q
            cmeta[c] = (slot_off - off_c, off_c)
        sb_meta.append({"blocks": blocks, "chunks": cmeta})
    TOT = slot_off
    TPB = Q // 128  # tiles per (block, chunk)

    blk_seq = [[] for _ in range(B)]
    for sbi, blocks in enumerate(sb_list):
        for c in range(NCH):
            _, off_c = sb_meta[sbi]["chunks"][c]
            for b in blocks:
                roff = regions[(b, c)]
                for t in range(TPB):
                    blk_seq[b].append(
                        (c, (roff - off_c) // 128 + t, roff // 128 + t))

    meta = {"B": B, "sb_meta": sb_meta, "blk_seq": blk_seq, "TOT": TOT}

    per_core = []
    for m in range(NC):
        cc = cores[m]
        idx_all = np.zeros(TOT, np.int16)
        r_all = np.zeros(TOT, np.float32)
        v_all = np.zeros(TOT, np.float32)
        for b in range(cc["nb"]):
            for c in range(NCH):
                s0, s1 = cc["starts"][b * NCH + c], cc["starts"][b * NCH + c + 1]
                if s1 == s0:
                    continue
                d0 = regions[(b, c)]
                idx_all[d0:d0 + s1 - s0] = cc["srcrel"][s0:s1]
                r_all[d0:d0 + s1 - s0] = cc["r"][s0:s1]
                v_all[d0:d0 + s1 - s0] = cc["v"][s0:s1]
        idx_w = np.ascontiguousarray(
            np.tile(idx_all.reshape(TOT // 16, 16).T, (8, 1)))
        r_w = np.ascontiguousarray(r_all.reshape(TOT // 128, 128).T)
        v_w = np.ascontiguousarray(v_all.reshape(TOT // 128, 128).T)
        # rowmap: real node n -> padded out row
        bstart = cc["bstart"]
        rowmap = np.empty(NS, np.int64)
        for b in range(cc["nb"]):
            n0, n1 = bstart[b], bstart[b + 1]
            rowmap[n0:n1] = b * BLK + np.arange(n1 - n0)
        per_core.append({"idx16": idx_w, "rarr": r_w, "varr": v_w,
                         "rowmap": rowmap})
    return meta, per_core


def _build_program(cfg, meta, bias_mode):
    import concourse.bacc as bacc
    import concourse.mybir as mybir
    import concourse.tile as tile

    dt = mybir.dt
    f32 = dt.float32
    NCH, CW, BLK, D = cfg.NCHUNK, cfg.CW, cfg.BLK, cfg.D
    NSP = meta["B"] * BLK
    TOT = meta["TOT"]

    nc = bacc.Bacc("TRN2", target_bir_lowering=False, debug=False,
                   num_devices=cfg.NCORES,
                   num_swdge_queues=getattr(cfg, "SWDGE_QUEUES", 1))

    x_d = nc.dram_tensor("x", [cfg.N, D], f32, kind="ExternalInput")
    idx_d = nc.dram_tensor("idx16", [128, TOT // 16], dt.int16,
                           kind="ExternalInput")
    r_d = nc.dram_tensor("rarr", [128, TOT // 128], f32, kind="ExternalInput")
    v_d = nc.dram_tensor("varr", [128, TOT // 128], f32, kind="ExternalInput")
    w_d = nc.dram_tensor("w", [D, D], f32, kind="ExternalInput")
    iota_d = nc.dram_tensor("iota", [128, 128], f32, kind="ExternalInput")
    ident_d = nc.dram_tensor("ident", [128, 128], f32, kind="ExternalInput")
    if bias_mode:
        bias_d = nc.dram_tensor("biasT", [D, NSP], f32, kind="ExternalInput")
    out_d = nc.dram_tensor("out", [NSP, D], f32, kind="ExternalOutput")

    Copy = mybir.ActivationFunctionType.Copy
    Relu = mybir.ActivationFunctionType.Relu
    EQ = mybir.AluOpType.is_equal
    MUL = mybir.AluOpType.mult

    with tile.TileContext(nc) as tc:
        with (
            tc.tile_pool(name="const", bufs=1) as cpool,
            tc.tile_pool(name="gather",
                         bufs=getattr(cfg, "GBUFS", 3)) as gpool,
            tc.tile_pool(name="ptile",
                         bufs=getattr(cfg, "PBUFS", 10)) as ppool,
            tc.tile_pool(name="epi", bufs=3) as epool,
            tc.tile_pool(name="acc", bufs=2, space="PSUM") as acc_pool,
            tc.tile_pool(name="tps", bufs=2, space="PSUM") as tps_pool,
        ):
            sidx = cpool.tile([128, TOT // 16], dt.int16, tag="sidx")
            sr = cpool.tile([128, TOT // 128], f32, tag="sr")
            sv = cpool.tile([128, TOT // 128], f32, tag="sv")
            sw = cpool.tile([D, D], f32, tag="sw")
            siota = cpool.tile([128, 128], f32, tag="siota")
            sident = cpool.tile([128, 128], f32, tag="sident")
            nc.sync.dma_start(sidx[:], idx_d[:])
            nc.sync.dma_start(sr[:], r_d[:])
            nc.sync.dma_start(sv[:], v_d[:])
            nc.sync.dma_start(sw[:], w_d[:])
            nc.sync.dma_start(siota[:], iota_d[:])
            nc.sync.dma_start(sident[:], ident_d[:])
            if bias_mode:
                sbias = cpool.tile([D, NSP], f32, tag="sbias")
                nc.sync.dma_start(sbias[:], bias_d[:])

            gq = [0]
            for sb in meta["sb_meta"]:
                gtiles = {}
                for c in range(NCH):
                    slots, off = sb["chunks"][c]
                    if slots == 0:
                        continue
                    g = gpool.tile([128, slots // 128, cfg.D], f32,
                                   tag=f"g{c}")
                    cap = getattr(cfg, "MAX_GATHER", 1 << 30)
                    nq = getattr(cfg, "SWDGE_QUEUES", 1)
                    sp = bool(getattr(cfg, "SINGLE_PACKET", True))
                    for p0 in range(0, slots, cap):
                        n = min(cap, slots - p0)
                        nc.gpsimd.dma_gather(
                            g[:, p0 // 128:(p0 + n) // 128, :],
                            x_d[c * CW:(c + 1) * CW, :],
                            sidx[:, (off + p0) // 16:(off + p0 + n) // 16],
                            n,
                            n,
                            cfg.D,
                            single_packet=sp,
                            queue_num=(gq[0] % nq),
                        )
                        gq[0] += 1
                    gtiles[c] = g
                for b in sb["blocks"]:
                    seq = meta["blk_seq"][b]
                    if not seq:
                        continue
                    ps = acc_pool.tile([BLK, D], f32, tag="ps")
                    act_every = getattr(cfg, "P_ACT_EVERY", 0)
                    for i, (c, col, gt) in enumerate(seq):
                        P = ppool.tile([128, 128], f32, tag="P")
                        if act_every and i % act_every == act_every - 1:
                            t1 = ppool.tile([128, 128], f32, tag="t1")
                            nc.scalar.activation(
                                t1[:], siota[:],
                                mybir.ActivationFunctionType.Abs,
                                bias=sr[:, gt:gt + 1], scale=-1.0)
                            nc.scalar.activation(
                                t1[:], t1[:], Relu, bias=1.0, scale=-1.0)
                            nc.scalar.activation(
                                P[:], t1[:], Copy, scale=sv[:, gt:gt + 1])
                        else:
                            nc.vector.tensor_scalar(
                                P[:], siota[:], sr[:, gt:gt + 1],
                                sv[:, gt:gt + 1], EQ, MUL)
                        nc.tensor.matmul(
                            ps[:], P[:], gtiles[c][:, col, :],
                            start=(i == 0), stop=(i == len(seq) - 1))
                    # epilogue: out_b = relu(agg @ W (+ deg*b))
                    s1 = epool.tile([BLK, D], f32, tag="s1")
                    nc.scalar.activation(s1[:], ps[:], Copy)
                    p2 = tps_pool.tile([D, BLK], f32, tag="p2")
                    nc.tensor.transpose(p2[:], s1[:], sident[:])
                    s2 = epool.tile([D, BLK], f32, tag="s2")
                    nc.scalar.activation(s2[:], p2[:], Copy)
                    p3 = tps_pool.tile([D, BLK], f32, tag="p3")
                    nc.tensor.matmul(p3[:], sw[:], s2[:],
                                     start=True, stop=True)
                    s3 = epool.tile([D, BLK], f32, tag="s3")
                    if bias_mode:
                        nc.vector.tensor_tensor(
                            s3[:], p3[:],
                            sbias[:, b * BLK:(b + 1) * BLK],
                            mybir.AluOpType.add)
                        nc.scalar.activation(s3[:], s3[:], Relu)
                    else:
                        nc.scalar.activation(s3[:], p3[:], Relu)
                    p4 = acc_pool.tile([BLK, D], f32, tag="p4")
                    nc.tensor.transpose(p4[:], s3[:], sident[:D, :D])
                    s4 = epool.tile([BLK, D], f32, tag="s4")
                    nc.scalar.activation(s4[:], p4[:], Copy)
                    nc.sync.dma_start(out_d[b * BLK:(b + 1) * BLK, :], s4[:])

    nc.compile()
    return nc


_CACHE = {}


def _get_program(cfg, meta, bias_mode):
    key = (id(cfg), meta["TOT"], meta["B"], bias_mode)
    if key not in _CACHE:
        _CACHE[key] = _build_program(cfg, meta, bias_mode)
    return _CACHE[key]


def kernel(x, adj_vals, W, b, edge_src, edge_dst, _cfg=None, _timing=None):
    from concourse.bass_utils import run_bass_kernel_spmd

    cfg = _cfg or CFG
    x = np.ascontiguousarray(np.asarray(x, np.float32))
    adj_vals = np.asarray(adj_vals, np.float32)
    W = np.ascontiguousarray(np.asarray(W, np.float32))
    b = np.asarray(b, np.float32)
    edge_src = np.asarray(edge_src, np.int64)
    edge_dst = np.asarray(edge_dst, np.int64)

    bias_mode = bool(np.any(b != 0))
    meta, per_core = _prepare(cfg, adj_vals, edge_src, edge_dst)
    nc = _get_program(cfg, meta, bias_mode)

    iota = np.tile(np.arange(128, dtype=np.float32), (128, 1))
    ident = np.eye(128, dtype=np.float32)
    NSP = meta["B"] * cfg.BLK

    in_maps = []
    for m in range(cfg.NCORES):
        im = {
            "x": x,
            "idx16": per_core[m]["idx16"],
            "rarr": per_core[m]["rarr"],
            "varr": per_core[m]["varr"],
            "w": W,
            "iota": iota,
            "ident": ident,
        }
        if bias_mode:
            deg = np.zeros(NSP, np.float32)
            sel = edge_dst // cfg.NS == m
            np.add.at(deg, per_core[m]["rowmap"][edge_dst[sel] - m * cfg.NS],
                      adj_vals[sel])
            im["biasT"] = np.ascontiguousarray(b[:, None] * deg[None, :])
        in_maps.append(im)

    res = run_bass_kernel_spmd(nc, in_maps, core_ids=list(range(cfg.NCORES)))
    if _timing is not None:
        for _ in range(_timing.get("iters", 2)):
            t0 = time.time()
            res = run_bass_kernel_spmd(
                nc, in_maps, core_ids=list(range(cfg.NCORES)))
            _timing.setdefault("wall_s", []).append(time.time() - t0)

    out = np.empty((cfg.N, cfg.D), np.float32)
    for m in range(cfg.NCORES):
        out[m * cfg.NS:(m + 1) * cfg.NS] = \
            res.results[m]["out"][per_core[m]["rowmap"]]
    return out
